# revision 12
# baseline (speedup 1.0000x reference)
"""CrossModalAttention Trainium2 kernel (fp8 DoubleRow version).

Math: with seq_len=1 on both sides, softmax over the single key is 1.0, so
MHA(q_in, kv_in) == (kv_in @ Wv.T + bv) @ out_w.T + out_b.  Folding on host:
    W = out_w @ Wv          c = bv @ out_w.T + out_b
gives   out_m = LayerNorm(kv @ W.T + c + residual) * g + b.

Device work per modality: one [B,1024]x[1024,1024] matmul + residual add +
LayerNorm.  Sharding: pure data parallel over the batch dim, 8 cores.

Implementation notes:
  - Matmul runs in fp8e4m3 with perf_mode=DoubleRow (K=256 per instruction,
    2x PE throughput).  Weights are pre-scaled by 64 so they stay in e4m3's
    normal range; residuals are pre-scaled by 64 to match (LayerNorm is
    scale-invariant, eps is scaled by 64^2 to stay exact).
  - The residual (+ folded bias c) is accumulated into PSUM by a bf16
    identity matmul, so the vector engine only does bn_stats.
  - ScalarE normalizes straight out of PSUM into bf16 output tiles.
  - Host pre-packs: transposed fp8 features [RT,P,2,KO,P], x64 bf16
    residuals [RT,P,2,D], weights W.T*64 chunked [P,KO,D], outputs come
    back as packed bf16 [RT,P,2,D] and are split/upcast on host.
"""

import numpy as np
import ml_dtypes

P = 128          # partitions
D = 1024         # hidden dim
KO = D // P      # 8 contraction chunks of 128
N_CORES = 8
B_FULL = 16384
B_CORE = B_FULL // N_CORES   # 2048
RT = B_CORE // P             # 16 row tiles per core
LN_EPS = 1e-5
WSCALE = 64.0

E4 = ml_dtypes.float8_e4m3
BF16 = ml_dtypes.bfloat16

_PROGRAM_CACHE = {}
_LAST_IN_MAPS = None


def _build_program(flags):
    """flags = (gb1, gb2): whether LayerNorm gamma/beta are nontrivial."""
    import contextlib
    import concourse.bass as bass
    import concourse.bacc as bacc
    import concourse.tile as tile
    from concourse import mybir
    from concourse.masks import make_identity
    from concourse._compat import get_trn_type

    gb1, gb2 = flags
    f32 = mybir.dt.float32
    bf = mybir.dt.bfloat16
    f8 = mybir.dt.float8e4
    DR = mybir.MatmulPerfMode.DoubleRow
    AF = mybir.ActivationFunctionType

    nc = bacc.Bacc(get_trn_type() or "TRN2", target_bir_lowering=False,
                   debug=False, num_devices=N_CORES)

    kvt = nc.dram_tensor("kvt", (RT, P, 2, KO, P), f8, kind="ExternalInput").ap()
    res = nc.dram_tensor("res", (RT, P, 2, D), bf, kind="ExternalInput").ap()
    w1 = nc.dram_tensor("w1", (P, KO, D), f8, kind="ExternalInput").ap()
    w2 = nc.dram_tensor("w2", (P, KO, D), f8, kind="ExternalInput").ap()
    aux_names = []
    if gb1:
        aux_names += ["g1", "b1"]
    if gb2:
        aux_names += ["g2", "b2"]
    aux = {n: nc.dram_tensor(n, (1, D), f32, kind="ExternalInput").ap()
           for n in aux_names}
    out = nc.dram_tensor("out", (RT, P, 2, D), bf, kind="ExternalOutput").ap()

    with tile.TileContext(nc) as tc:
        with contextlib.ExitStack() as ctx:
            const = ctx.enter_context(tc.tile_pool(name="const", bufs=1))
            kvtp = ctx.enter_context(tc.tile_pool(name="kvtp", bufs=6))
            resp = ctx.enter_context(tc.tile_pool(name="resp", bufs=6))
            outp = ctx.enter_context(tc.tile_pool(name="outp", bufs=3))
            statp = ctx.enter_context(tc.tile_pool(name="statp", bufs=8))
            psum = ctx.enter_context(
                tc.tile_pool(name="psum", bufs=4, space="PSUM"))

            ident = const.tile([P, P], bf, tag="ident")
            make_identity(nc, ident)
            eps = const.tile([P, 1], f32, tag="eps")
            nc.vector.memset(eps, LN_EPS * WSCALE * WSCALE)

            # Warmup matmuls: keep the PE busy during the initial DMA fill so
            # the HAM clock-gate reaches 2.4 GHz before the real work starts.
            ps_warm = psum.tile([P, 2, 512], f32, tag="ps", name="ps_warm")
            for wi in range(24):
                nc.tensor.matmul(ps_warm[:, 0, 0:P], ident, ident,
                                 start=True, stop=True)

            w_t = {1: const.tile([P, KO, D], f8, tag="w1", name="w1t"),
                   2: const.tile([P, KO, D], f8, tag="w2", name="w2t")}

            def load_rt(rt):
                kv_t = kvtp.tile([P, 2, KO, P], f8, tag="kvt")
                nc.sync.dma_start(kv_t, kvt[rt])
                r_t = resp.tile([P, 2, D], bf, tag="res")
                nc.sync.dma_start(r_t, res[rt])
                return kv_t, r_t

            # DMA head order: first row tile's operands and the first weight
            # chunks lead so matmuls start ASAP; remaining weight chunks are
            # interleaved with the next row tiles' features.
            prefetched = {}
            kv0 = kvtp.tile([P, 2, KO, P], f8, tag="kvt", name="kv0")
            r0 = resp.tile([P, 2, D], bf, tag="res", name="r0")
            # modality-0 operands of row tile 0 first, at half granularity,
            # so the very first matmul block starts as early as possible
            nc.sync.dma_start(kv0[:, 0], kvt[0][:, 0])
            nc.sync.dma_start(w_t[1][:, 0:2, :], w1[:, 0:2, :])
            nc.sync.dma_start(r0[:, 0, :], res[0][:, 0, :])
            nc.sync.dma_start(kv0[:, 1], kvt[0][:, 1])
            nc.sync.dma_start(r0[:, 1, :], res[0][:, 1, :])
            prefetched[0] = (kv0, r0)
            nc.sync.dma_start(w_t[1][:, 2:4, :], w1[:, 2:4, :])
            nc.sync.dma_start(w_t[1][:, 4:6, :], w1[:, 4:6, :])
            nc.sync.dma_start(w_t[1][:, 6:8, :], w1[:, 6:8, :])
            nc.sync.dma_start(w_t[2][:, 0:2, :], w2[:, 0:2, :])
            prefetched[1] = load_rt(1)
            nc.sync.dma_start(w_t[2][:, 2:4, :], w2[:, 2:4, :])
            nc.sync.dma_start(w_t[2][:, 4:6, :], w2[:, 4:6, :])
            prefetched[2] = load_rt(2)
            nc.sync.dma_start(w_t[2][:, 6:8, :], w2[:, 6:8, :])
            prefetched[3] = load_rt(3)

            # broadcast-replicated aux rows ([1, D] dram -> [P, D] sbuf)
            aux_sb = {}
            for n, ap in aux.items():
                t = const.tile([P, D], f32, tag=n)
                bcast = bass.AP(tensor=ap.tensor, offset=ap.offset,
                                ap=[[0, P], ap.ap[1]])
                nc.sync.dma_start(t, bcast)
                aux_sb[n] = t

            for rt in range(RT):
                kv_t, r_t = prefetched.pop(rt)
                if rt + 4 < RT:
                    prefetched[rt + 4] = load_rt(rt + 4)
                o_t = outp.tile([P, 2, D], bf, tag="o")
                for m, gbk in ((0, gb1), (1, gb2)):
                    w = w_t[m + 1]
                    ps = psum.tile([P, 2, 512], f32, tag="ps")
                    for j in range(0, KO, 2):
                        for nh in range(2):
                            nc.tensor.matmul(
                                ps[:, nh, :],
                                kv_t[:, m, j:j + 2, :],
                                w[:, j:j + 2, nh * 512:(nh + 1) * 512],
                                start=(j == 0), stop=False,
                                perf_mode=DR)
                    for nh in range(2):
                        nc.tensor.matmul(
                            ps[:, nh, :], ident,
                            r_t[:, m, nh * 512:(nh + 1) * 512],
                            start=False, stop=True)

                    stats = statp.tile([P, 2, 6], f32, tag="stats")
                    nc.vector.bn_stats(stats[:, 0, :], ps[:, 0, :])
                    nc.vector.bn_stats(stats[:, 1, :], ps[:, 1, :])
                    mv = statp.tile([P, 2], f32, tag="mv")
                    nc.vector.bn_aggr(mv, stats)
                    # mv[:,0]=mean, mv[:,1]=var (of 64*s); rstd64 then
                    # nb = -mu * rstd so ACT computes (s*rstd + nb)
                    nc.scalar.activation(
                        out=mv[:, 1:2], in_=mv[:, 1:2],
                        func=AF.Sqrt, bias=eps, scale=1.0)
                    nc.vector.reciprocal(mv[:, 1:2], mv[:, 1:2])
                    nb = statp.tile([P, 1], f32, tag="nb")
                    nc.vector.tensor_scalar(
                        out=nb, in0=mv[:, 0:1],
                        scalar1=mv[:, 1:2], scalar2=-1.0,
                        op0=mybir.AluOpType.mult,
                        op1=mybir.AluOpType.mult)
                    nc.scalar.activation(
                        out=o_t[:, m, :], in_=ps,
                        func=AF.Identity,
                        bias=nb, scale=mv[:, 1:2])
                    if gbk:
                        gk, bk = (f"g{m + 1}", f"b{m + 1}")
                        nc.vector.tensor_mul(
                            out=o_t[:, m, :], in0=o_t[:, m, :], in1=aux_sb[gk])
                        nc.vector.tensor_add(
                            out=o_t[:, m, :], in0=o_t[:, m, :], in1=aux_sb[bk])
                nc.sync.dma_start(out[rt], o_t)

    nc.compile()
    return nc


def _fold(in_w, in_b, out_w, out_b):
    Dv = out_w.shape[0]
    Wv = in_w[2 * Dv:3 * Dv, :].astype(np.float64)
    bv = in_b[2 * Dv:3 * Dv].astype(np.float64)
    W = (out_w.astype(np.float64) @ Wv).astype(np.float32)
    c = (bv @ out_w.astype(np.float64).T + out_b.astype(np.float64)
         ).astype(np.float32)
    return W, c


def _pack_w(W):
    # w[p, j, n] = (64*W)[n, j*128+p]  == chunked W.T, fp8
    w64 = np.clip(W.astype(np.float64) * WSCALE, -224.0, 224.0)
    wt = np.ascontiguousarray(
        w64.T.reshape(KO, P, D).transpose(1, 0, 2)).astype(E4)
    return wt


def kernel(image_features, text_features,
           in_w1, in_b1, out_w1, out_b1,
           in_w2, in_b2, out_w2, out_b2,
           ln1_g, ln1_b, ln2_g, ln2_b):
    from concourse import bass_utils

    img = np.ascontiguousarray(image_features, dtype=np.float32)
    txt = np.ascontiguousarray(text_features, dtype=np.float32)

    W1, c1 = _fold(np.asarray(in_w1), np.asarray(in_b1),
                   np.asarray(out_w1), np.asarray(out_b1))
    W2, c2 = _fold(np.asarray(in_w2), np.asarray(in_b2),
                   np.asarray(out_w2), np.asarray(out_b2))
    w1p, w2p = _pack_w(W1), _pack_w(W2)

    # fp8 copies of features (matmul operands)
    img8 = np.clip(img, -224, 224).astype(E4)
    txt8 = np.clip(txt, -224, 224).astype(E4)
    # x64 bf16 residuals with the folded bias absorbed
    res_img = ((img + c1[None, :]) * WSCALE).astype(BF16)
    res_txt = ((txt + c2[None, :]) * WSCALE).astype(BF16)

    flags = (
        bool(np.any(np.asarray(ln1_g) != 1) or np.any(np.asarray(ln1_b))),
        bool(np.any(np.asarray(ln2_g) != 1) or np.any(np.asarray(ln2_b))),
    )

    if flags not in _PROGRAM_CACHE:
        _PROGRAM_CACHE[flags] = _build_program(flags)
    nc = _PROGRAM_CACHE[flags]

    in_maps = []
    for c in range(N_CORES):
        rows = slice(c * B_CORE, (c + 1) * B_CORE)
        kvt = np.empty((RT, P, 2, KO, P), E4)
        # kvt[rt, p, m, j, b] = X[rt*128+b, j*128+p]
        kvt[:, :, 0] = txt8[rows].reshape(RT, P, KO, P).transpose(0, 3, 2, 1)
        kvt[:, :, 1] = img8[rows].reshape(RT, P, KO, P).transpose(0, 3, 2, 1)
        resm = np.empty((RT, P, 2, D), BF16)
        resm[:, :, 0] = res_img[rows].reshape(RT, P, D)
        resm[:, :, 1] = res_txt[rows].reshape(RT, P, D)
        m = {
            "kvt": np.ascontiguousarray(kvt),
            "res": np.ascontiguousarray(resm),
            "w1": w1p,
            "w2": w2p,
        }
        if flags[0]:
            m["g1"] = np.asarray(ln1_g, np.float32).reshape(1, D)
            m["b1"] = np.asarray(ln1_b, np.float32).reshape(1, D)
        if flags[1]:
            m["g2"] = np.asarray(ln2_g, np.float32).reshape(1, D)
            m["b2"] = np.asarray(ln2_b, np.float32).reshape(1, D)
        in_maps.append(m)

    global _LAST_IN_MAPS
    _LAST_IN_MAPS = in_maps
    res_r = bass_utils.run_bass_kernel_spmd(nc, in_maps, list(range(N_CORES)))
    outs = [np.asarray(res_r.results[c]["out"]).reshape(B_CORE, 2, D)
            for c in range(N_CORES)]
    full = np.concatenate(outs, axis=0)
    attended_image = full[:, 0, :].astype(np.float32)
    attended_text = full[:, 1, :].astype(np.float32)
    return attended_image, attended_text
